# revision 9
# baseline (speedup 1.0000x reference)
"""Batched CRF Viterbi decode on 8 TRN2 NeuronCores.

Sharding: data-parallel over batch (16 sequences per core), transitions
replicated. The sequential forward max-plus recurrence runs on-device;
backpointer reconstruction + backtrack run on host from the partition
history.

The recurrence is recentered per (b, t): c[b,t] = max_j feats[b,t,j] is
subtracted from feats on the host, so the running state (resid) stays in
[-17, 12] instead of drifting to ~1100. That lets the state and the
per-step score tile travel through the PE array as float16 (1 cycle/row
vs 4 for float32) with ~2^-11 rounding, which flips ~10 of 65536 decoded
tags vs the fp32 reference (rel err ~1e-2, inside the 2e-2 gate). The
decode is shift-invariant, so the host backtrack consumes the recentered
history directly.

Device layout (per core, BL=16 sequences):
  partitions p = jg*16 + b  (8 j-groups x 16 batch), tag j = jg*7 + jl
  Per step t:
    PSUM C[p,(jl,i)] = fl(ft16 + resid16):
      ft16 = fp16(trans[i,j] + feats'[b,t,j]) built by Pool in SBUF,
      moved to PSUM via an fp16 identity matmul; resid via an fp16 K=128
      matmul (REP128 @ zero-embedded state) accumulating on top.
    M[p,jl] = max_i C  (VectorE 3D reduce) -> fp16 hist column
    mfw[p,i] = M-bcast * G  (zero-embed so the REP128 matmul can
      reassemble resid[b,i] by summing the 8 jg partitions)
"""

import numpy as np

B, S, T = 128, 512, 50
NCORES = 8
BL = B // NCORES          # 16 sequences per core
JG, JL = 8, 7             # 8 j-groups x 7 tags each = 56 padded tags
TP = JG * JL              # 56
NF = JL * TP              # 392 psum columns (only 7x50=350 are live)
START, END = T - 2, T - 1
NEG = np.float32(-25000.0)  # padding; must survive float16 (|x| < 65504)


def _host_prep(feats, transitions):
    """Build per-core device input arrays."""
    f = np.ascontiguousarray(feats, dtype=np.float32)         # (B,S,T)
    tr = np.ascontiguousarray(transitions, dtype=np.float32)  # (T,T)
    c = f.max(axis=2)                                         # (B,S) recenter

    trp = np.full((TP, TP), NEG, dtype=np.float32)
    trp[:T, :T] = tr

    k = np.arange(128)
    # transP[p=(jg,b), (jl, i)] = trp[i, jg*7+jl]
    transP = np.empty((128, JL, TP), dtype=np.float32)
    for g in range(JG):
        transP[g * BL:(g + 1) * BL] = trp[:, g * JL:(g + 1) * JL].T[None]
    transP = np.ascontiguousarray(transP.reshape(128, NF))

    # REP128[k, m] = 1 if k%16 == m%16  (sum over jg of zero-embedded state)
    REP128 = (k[:, None] % BL == k[None, :] % BL).astype(np.float16)
    # G[p, i] = 1 if i//7 == p//16 else 0   (zero-embed mask)
    G = ((np.arange(TP)[None, :] // JL) == (k[:, None] // BL)).astype(np.float32)

    per_core = []
    for c_id in range(NCORES):
        fb = f[c_id * BL:(c_id + 1) * BL]                     # (16,S,T)
        cb = c[c_id * BL:(c_id + 1) * BL]                     # (16,S)
        # feats_arr[p=(jg,b), t*7+jl] = feats[b,t,jg*7+jl] - c[b,t]
        fp = np.zeros((BL, S, TP), dtype=np.float32)
        fp[:, :, :T] = fb - cb[:, :, None]
        fa = fp.reshape(BL, S, JG, JL).transpose(2, 0, 1, 3).reshape(128, S * JL)

        part0 = np.full((BL, TP), NEG, dtype=np.float32)
        part0[:, :T] = fb[:, 0, :] + tr[START][None, :] - cb[:, 0:1]
        mw0 = np.repeat(part0[None, :, :], JG, axis=0).reshape(128, TP) * G
        per_core.append({
            "feats_arr": np.ascontiguousarray(fa),
            "transP": transP,
            "I128": np.eye(128, dtype=np.float16),
            "REP128": REP128,
            "G": G,
            "mw0": np.ascontiguousarray(mw0.astype(np.float16)),
        })
    return per_core


def build_bass(n_steps):
    import concourse.bacc as bacc
    import concourse.mybir as mybir
    import concourse.tile as tile

    f32 = mybir.dt.float32
    f16 = mybir.dt.float16
    nc = bacc.Bacc("TRN2", target_bir_lowering=False, debug=False,
                   num_devices=NCORES)

    feats_d = nc.declare_dram_parameter("feats_arr", [128, S * JL], f32, isOutput=False)
    transP_d = nc.declare_dram_parameter("transP", [128, NF], f32, isOutput=False)
    i128_d = nc.declare_dram_parameter("I128", [128, 128], f16, isOutput=False)
    rep_d = nc.declare_dram_parameter("REP128", [128, 128], f16, isOutput=False)
    g_d = nc.declare_dram_parameter("G", [128, TP], f32, isOutput=False)
    mw0_d = nc.declare_dram_parameter("mw0", [128, TP], f16, isOutput=False)
    hist_d = nc.declare_dram_parameter("hist", [128, n_steps * JL], f32, isOutput=True)

    with tile.TileContext(nc) as tc:
        with (
            tc.tile_pool(name="static", bufs=1) as sp,
            tc.tile_pool(name="state", bufs=6) as st,
            tc.tile_pool(name="psum", bufs=3, space="PSUM") as pp,
        ):
            feats_sb = sp.tile([128, S * JL], f32)
            # chunked so early steps start before the whole tensor lands
            fchunk = S * JL // 4
            for ci in range(4):
                nc.sync.dma_start(
                    out=feats_sb[:, ci * fchunk:(ci + 1) * fchunk],
                    in_=feats_d[:, ci * fchunk:(ci + 1) * fchunk])
            transP_sb = sp.tile([128, NF], f32)
            nc.sync.dma_start(out=transP_sb[:, :], in_=transP_d[:, :])
            i128_sb = sp.tile([128, 128], f16)
            nc.sync.dma_start(out=i128_sb[:, :], in_=i128_d[:, :])
            rep_sb = sp.tile([128, 128], f16)
            nc.sync.dma_start(out=rep_sb[:, :], in_=rep_d[:, :])
            g_sb = sp.tile([128, JG, JL], f32)
            nc.sync.dma_start(out=g_sb[:, :, :], in_=g_d[:, :].rearrange(
                "p (a b) -> p a b", a=JG))

            hist_sb = sp.tile([128, n_steps * JL], f32)

            mfw = st.tile([128, TP], f16, tag="mfw")
            nc.sync.dma_start(out=mfw[:, :], in_=mw0_d[:, :])

            transP_v = transP_sb[:, :].rearrange("p (a b) -> p a b", a=JL)

            ft_tiles = {}

            def build_ft(tt):
                # ft16 = fp16(trans + feats'): statics only, issued steps
                # ahead so it lands off the critical chain
                ft = st.tile([128, JL, T], f16, tag="ft%d" % (tt % 2))
                nc.gpsimd.tensor_tensor(
                    out=ft[:, :, :],
                    in0=transP_v[:, :, :T],
                    in1=feats_sb[:, tt * JL:(tt + 1) * JL].unsqueeze(2)
                    .broadcast_to([128, JL, T]),
                    op=mybir.AluOpType.add)
                ft_tiles[tt] = ft

            from concourse.tile_rust import add_dep_helper

            c_tiles = {}

            def issue_ftmm(tt, after=None):
                # C = ft16 via identity matmul as a COMPLETE group
                # (start+stop); the REP matmul later accumulates the state
                # on top (start=False). fp16 operands run the PE at 1
                # cycle/row.
                c_ps = pp.tile([128, JL, T], f32, tag="C%d" % (tt % 2))
                mm = nc.tensor.matmul(
                    c_ps[:, :, :], i128_sb[:, :], ft_tiles.pop(tt)[:, :, :],
                    start=True, stop=True)
                if after is not None:
                    # ordering-only edge: schedule this matmul globally after
                    # the state mult, so Tile's wait-emission pass gates the
                    # reduce on the REP matmul (its true dep), not on this one
                    add_dep_helper(
                        mm.ins, after.ins, sync=False,
                        reason="FT-mm after state mult (wait precision)")
                c_tiles[tt] = c_ps

            for tt in range(1, min(4, n_steps + 1)):
                build_ft(tt)
            issue_ftmm(1)
            for t in range(1, n_steps + 1):
                if t + 3 <= n_steps:
                    build_ft(t + 3)
                HI = 4 * JL  # i-split: groups 0-3 (28 cols) / 4-7 (22 live)
                c_ps = c_tiles.pop(t)
                nc.tensor.matmul(
                    c_ps[:, :, :HI], rep_sb[:, :],
                    mfw[:, :HI].unsqueeze(1).broadcast_to([128, JL, HI]),
                    start=False, stop=True, skip_group_check=True)
                nc.tensor.matmul(
                    c_ps[:, :, HI:], rep_sb[:, :],
                    mfw[:, HI:T].unsqueeze(1).broadcast_to([128, JL, T - HI]),
                    start=False, stop=True, skip_group_check=True)

                m_col = hist_sb[:, (t - 1) * JL: t * JL]
                nc.vector.tensor_reduce(
                    m_col, c_ps[:, :, :],
                    axis=mybir.AxisListType.X, op=mybir.AluOpType.max)

                mfw = st.tile([128, TP], f16, tag="mfw")
                nc.vector.tensor_tensor(
                    out=mfw[:, :HI].rearrange("p (a b) -> p a b", a=4),
                    in0=m_col.unsqueeze(1).broadcast_to([128, 4, JL]),
                    in1=g_sb[:, :4, :],
                    op=mybir.AluOpType.mult)
                mult_inst = nc.vector.tensor_tensor(
                    out=mfw[:, HI:].rearrange("p (a b) -> p a b", a=4),
                    in0=m_col.unsqueeze(1).broadcast_to([128, 4, JL]),
                    in1=g_sb[:, 4:, :],
                    op=mybir.AluOpType.mult)

                if t + 1 <= n_steps:
                    issue_ftmm(t + 1, after=mult_inst)

                # drain finished quarters of the history while computing
                if t % 128 == 0 and t < n_steps:
                    lo, hi = (t - 128) * JL, t * JL
                    nc.sync.dma_start(out=hist_d[:, lo:hi],
                                      in_=hist_sb[:, lo:hi])

            done = (n_steps // 128) * 128 * JL if n_steps >= 128 else 0
            if n_steps * JL > done:
                nc.sync.dma_start(out=hist_d[:, done:n_steps * JL],
                                  in_=hist_sb[:, done:n_steps * JL])

    nc.compile()
    return nc


def device_model(inp, n_steps):
    """Numpy model of the device kernel (for validation)."""
    fa = inp["feats_arr"]
    transP = inp["transP"]
    REP128 = inp["REP128"].astype(np.float32)
    G = inp["G"].astype(np.float32)
    mfw = inp["mw0"].astype(np.float32)
    hist = np.zeros((128, n_steps * JL), dtype=np.float32)
    for t in range(1, n_steps + 1):
        ft = transP.reshape(128, JL, TP)[:, :, :T] + np.repeat(
            fa[:, t * JL:(t + 1) * JL], T, axis=1).reshape(128, JL, T)
        ft16 = ft.astype(np.float16).astype(np.float32)
        C = ft16 + np.broadcast_to(
            (REP128.T @ mfw[:, :T])[:, None, :], (128, JL, T))
        M = C.max(axis=2).astype(np.float32)
        hist[:, (t - 1) * JL: t * JL] = M
        m16 = M.astype(np.float16).astype(np.float32)
        mfw = (np.broadcast_to(m16[:, None, :], (128, JG, JL)).reshape(128, TP)
               * G).astype(np.float16).astype(np.float32)
    return hist


def viterbi_host(part_hist, feats, mask, transitions):
    """Backpointer reconstruction + backtrack from the recentered history
    (the decode is invariant to the per-(b,t) shift)."""
    Bv = feats.shape[0]
    lengths = mask.astype(np.int64).sum(axis=1)
    last_pos = lengths - 1
    bidx = np.arange(Bv)

    last_part = part_hist[last_pos, bidx]                     # (B, T)
    last_values = last_part[:, :, None] + transitions[None]   # (B, i, j)
    pointer = np.argmax(last_values[:, :, END], axis=1).astype(np.int32)

    decode = np.zeros((S, Bv), dtype=np.int32)
    decode[S - 1] = pointer
    ptr = pointer.copy()
    trT = np.ascontiguousarray(transitions.T)                 # (j, i)
    for t in range(S - 2, -1, -1):
        sc = feats[bidx, t + 1, ptr][:, None] + trT[ptr]      # (B, i)
        cur = sc + part_hist[t]                               # (B, i)
        bp = np.argmax(cur, axis=1).astype(np.int32)
        bp = np.where(mask[:, t + 1], bp, 0)
        at_last = last_pos == t
        new_ptr = np.where(at_last, pointer, bp).astype(np.int32)
        decode[t] = new_ptr
        ptr = new_ptr
    return decode.T


def reassemble_part_hist(results, feats, transitions):
    f = np.asarray(feats, dtype=np.float32)
    c = f.max(axis=2)                                         # (B,S)
    part_hist = np.empty((S, B, T), dtype=np.float32)
    part_hist[0] = f[:, 0, :] + transitions[START][None, :] - c[:, 0:1]
    for cid in range(NCORES):
        hist = results[cid]["hist"].astype(np.float32)        # (128, 511*7)
        h = hist.reshape(JG, BL, S - 1, JL).transpose(2, 1, 0, 3)
        part_hist[1:, cid * BL:(cid + 1) * BL, :] = \
            h.reshape(S - 1, BL, TP)[:, :, :T]
    return part_hist


def kernel(feats, mask, transitions):
    from concourse.bass_utils import run_bass_kernel_spmd

    feats = np.asarray(feats, dtype=np.float32)
    mask_np = np.asarray(mask).astype(bool)
    transitions = np.asarray(transitions, dtype=np.float32)

    per_core = _host_prep(feats, transitions)
    nc = build_bass(S - 1)
    res = run_bass_kernel_spmd(nc, per_core, core_ids=list(range(NCORES)))

    part_hist = reassemble_part_hist(res.results, feats, transitions)
    return viterbi_host(part_hist, feats, mask_np, transitions).astype(np.int32)
